# revision 12
# baseline (speedup 1.0000x reference)
"""GCN (3-layer) + dueling-DQN head on 8 Trainium2 NeuronCores.

Sharding: nodes dst-sharded 6250/core. Per layer, per core:
  dense T = h @ W (feat-major on PE), transpose local shard to node-major
  bf16 rows -> DRAM table, AllGather table shards across cores, then
  per-edge dma_gather (HBM->SBUF, 256B rows) + PE scatter-matmul
  (lhsT = gathered messages [128 edges x 128 feat], rhs = indicator
  [128 edges x 32 dst-window] whose values are the GCN edge norms
  dinv[src]*dinv[dst], self-loops as dinv^2 edges) accumulating into
  PSUM feat-major, relu+bias evict. Mean-pool partials per core; host
  combines partials and applies the tiny dueling heads (unshard step).
"""
import numpy as np
import ml_dtypes

bf16 = ml_dtypes.bfloat16

N_NODES = 50000
N_EDGES = 800000
F_IN = 11
HID = 128
N_ACT = 6
NCORES = 8
SHARD = N_NODES // NCORES          # 6250
HALF = 25000                       # (unused) old half boundary
HALFROWS = 3200                    # local rows per core in table half A (25 tiles)
NTILE_A = HALFROWS // 128          # 25
TBL_A = NCORES * HALFROWS          # 25600
TBL_B = NCORES * (SHARD - HALFROWS)  # 24400
WIN = 32                           # dsts per indicator window
WINDOWS = 196                      # ceil(6272/32); SHARD padded to 6272
CHUNK_WINS = 14                    # windows per PSUM bank chunk
CHUNKS = WINDOWS // CHUNK_WINS     # 14
DPAD = 6272                        # padded local dst count (49*128)
NTILE = DPAD // 128                # 49 node tiles


def _prep(edge_index):
    """Host-side sharding/prep. Returns uniform-shape per-core index data +
    the static block schedule (identical across cores; SPMD requirement)."""
    src = np.asarray(edge_index[0], dtype=np.int64)
    dst = np.asarray(edge_index[1], dtype=np.int64)
    deg = np.bincount(dst, minlength=N_NODES).astype(np.float64) + 1.0
    dinv = (1.0 / np.sqrt(deg)).astype(np.float32)

    cores = []
    for c in range(NCORES):
        m = (dst // SHARD) == c
        es = src[m]
        ed = dst[m] - c * SHARD
        w = dinv[src[m]] * dinv[dst[m]]
        vs = np.arange(SHARD, dtype=np.int64) + c * SHARD
        es = np.concatenate([es, vs])
        ed = np.concatenate([ed, np.arange(SHARD, dtype=np.int64)])
        w = np.concatenate([w, dinv[vs] * dinv[vs]]).astype(np.float32)
        ec = es // SHARD
        er = es - ec * SHARD
        half = (er >= HALFROWS).astype(np.int64)
        lidx = np.where(half == 0, ec * HALFROWS + er,
                        ec * (SHARD - HALFROWS) + (er - HALFROWS))
        win = ed // WIN
        col = ed - win * WIN
        order = np.lexsort((lidx, half, win))
        cores.append((lidx[order], half[order], win[order], col[order], w[order]))

    # uniform (max-over-cores) padded segment length per (window, half)
    seglen = np.zeros((WINDOWS, 2), dtype=np.int64)
    for (lidx, half, win, col, w) in cores:
        for h in (0, 1):
            cnt = np.bincount(win[half == h], minlength=WINDOWS)
            seglen[:, h] = np.maximum(seglen[:, h], cnt)
    padlen = ((seglen + 127) // 128) * 128  # slots per (window, half)

    # per-chunk call sizes and block schedule (uniform)
    chunk_meta = []   # per chunk: dict(n0, n1, sched=[(win_in_chunk, buf_row, blk_id, first, last)])
    blk_id = 0
    for k in range(CHUNKS):
        ws = range(k * CHUNK_WINS, (k + 1) * CHUNK_WINS)
        n0 = int(sum(padlen[w, 0] for w in ws))
        n1 = int(sum(padlen[w, 1] for w in ws))
        sched = []
        off0 = 0
        off1 = n0
        for wi, w in enumerate(ws):
            blocks = []
            nb0 = padlen[w, 0] // 128
            nb1 = padlen[w, 1] // 128
            for j in range(nb0):
                blocks.append(off0 // 128 + j)
            off0 += padlen[w, 0]
            for j in range(nb1):
                blocks.append(off1 // 128 + j)
            off1 += padlen[w, 1]
            for bi, brow in enumerate(blocks):
                sched.append((wi, int(brow), blk_id,
                              bi == 0, bi == len(blocks) - 1))
                blk_id += 1
        chunk_meta.append(dict(n0=n0, n1=n1, sched=sched,
                               ctot=(n0 + n1) // 128))
    nblocks = blk_id
    cmax = max(m["ctot"] for m in chunk_meta)

    # fill per-core slot data: gather idx streams (per half) + indicator
    L0 = int(sum(m["n0"] for m in chunk_meta))
    L1 = int(sum(m["n1"] for m in chunk_meta))
    per_core = []
    for (lidx, half, win, col, w) in cores:
        idx0 = np.zeros(L0, dtype=np.int16)
        idx1 = np.zeros(L1, dtype=np.int16)
        ind = np.zeros((nblocks, 128, WIN), dtype=bf16)  # [blk, slot, col]
        o0 = o1 = 0
        blk = 0
        for k in range(CHUNKS):
            ws = range(k * CHUNK_WINS, (k + 1) * CHUNK_WINS)
            # half-0 region then half-1 region; indicator blocks must be
            # emitted in schedule order == (window -> h0 blocks -> h1 blocks)
            seg_data = {}
            for w_ in ws:
                for h in (0, 1):
                    sel = (win == w_) & (half == h)
                    seg_data[(w_, h)] = (lidx[sel], col[sel], w[sel])
            # write idx streams (buffer order: all h0 segs, then all h1 segs)
            for h, (oo, idx_arr) in ((0, (o0, idx0)), (1, (o1, idx1))):
                o = oo
                for w_ in ws:
                    li, co, va = seg_data[(w_, h)]
                    n = len(li)
                    idx_arr[o:o + n] = li.astype(np.int16)
                    o += padlen[w_, h]
                if h == 0:
                    o0 = o
                else:
                    o1 = o
            # indicator blocks in schedule order
            for w_ in ws:
                for h in (0, 1):
                    li, co, va = seg_data[(w_, h)]
                    nb = padlen[w_, h] // 128
                    for j in range(nb):
                        sl = slice(j * 128, min((j + 1) * 128, len(li)))
                        cs = co[sl]
                        vs_ = va[sl]
                        rows = np.arange(len(cs))
                        ind[blk, rows, cs] = vs_.astype(bf16)
                        blk += 1
        assert blk == nblocks
        # idx arrays wrapped in 16 partitions: slot i -> [i % 16, i // 16]
        i0w = np.tile(idx0.reshape(L0 // 16, 16).T, (8, 1)).copy()
        i1w = np.tile(idx1.reshape(L1 // 16, 16).T, (8, 1)).copy()
        # indicator SBUF layout [128 slots, nblocks*WIN]
        indw = np.ascontiguousarray(ind.transpose(1, 0, 2)).reshape(128, nblocks * WIN)
        per_core.append(dict(idx0=i0w, idx1=i1w, ind=indw))

    return dict(chunk_meta=chunk_meta, cmax=cmax, nblocks=nblocks,
                L0=L0, L1=L1, per_core=per_core, dinv=dinv)


def _build(prep):
    import concourse.bass as bass
    import concourse.bacc as bacc
    import concourse.mybir as mybir
    from concourse import tile

    dt = mybir.dt
    nc = bacc.Bacc("TRN2", target_bir_lowering=False, debug=False,
                   num_devices=NCORES, num_swdge_queues=4)

    L0, L1, nblocks, cmax = prep["L0"], prep["L1"], prep["nblocks"], prep["cmax"]
    chunk_meta = prep["chunk_meta"]

    # I/O
    xT = nc.dram_tensor("xT", [F_IN, DPAD], dt.float32, kind="ExternalInput")
    W1 = nc.dram_tensor("W1", [F_IN, HID], dt.float32, kind="ExternalInput")
    W2 = nc.dram_tensor("W2", [HID, HID], dt.bfloat16, kind="ExternalInput")
    W3 = nc.dram_tensor("W3", [HID, HID], dt.bfloat16, kind="ExternalInput")
    bcols = nc.dram_tensor("bcols", [HID, 3], dt.float32, kind="ExternalInput")
    idx0 = nc.dram_tensor("idx0", [128, L0 // 16], dt.int16, kind="ExternalInput")
    idx1 = nc.dram_tensor("idx1", [128, L1 // 16], dt.int16, kind="ExternalInput")
    indt = nc.dram_tensor("indt", [128, nblocks * WIN], dt.bfloat16,
                          kind="ExternalInput")
    ident = nc.dram_tensor("ident", [128, 128], dt.bfloat16, kind="ExternalInput")
    gsum = nc.dram_tensor("gsum", [HID, 1], dt.float32, kind="ExternalOutput")

    with tile.TileContext(nc) as tc:
        with (
            tc.tile_pool(name="const", bufs=1) as cpool,
            tc.tile_pool(name="h", bufs=2) as hpool,
            tc.tile_pool(name="tbf", bufs=1) as tpool,
            tc.tile_pool(name="stage", bufs=1) as spool,
            tc.tile_pool(name="msg", bufs=3) as mpool,
            tc.tile_pool(name="indp", bufs=3) as ipool,
            tc.tile_pool(name="pd", bufs=2, space="PSUM") as pdpool,
            tc.tile_pool(name="pt", bufs=2, space="PSUM") as ptpool,
            tc.tile_pool(name="ps", bufs=3, space="PSUM") as pspool,
            tc.tile_pool(name="dram", bufs=1, space="DRAM") as dpool,
        ):
            # resident constants
            xT_s = cpool.tile([F_IN, DPAD], dt.float32, tag="xT")
            nc.sync.dma_start(xT_s[:], xT[:])
            W1_s = cpool.tile([F_IN, HID], dt.float32, tag="W1")
            nc.sync.dma_start(W1_s[:], W1[:])
            W2_s = cpool.tile([HID, HID], dt.bfloat16, tag="W2")
            nc.sync.dma_start(W2_s[:], W2[:])
            W3_s = cpool.tile([HID, HID], dt.bfloat16, tag="W3")
            nc.sync.dma_start(W3_s[:], W3[:])
            b_s = cpool.tile([HID, 3], dt.float32, tag="b")
            nc.sync.dma_start(b_s[:], bcols[:])
            idx0_s = cpool.tile([128, L0 // 16], dt.int16, tag="idx0")
            nc.sync.dma_start(idx0_s[:], idx0[:])
            idx1_s = cpool.tile([128, L1 // 16], dt.int16, tag="idx1")
            nc.sync.dma_start(idx1_s[:], idx1[:])
            bmax = max(len(m["sched"]) for m in chunk_meta)
            id_s = cpool.tile([128, 128], dt.bfloat16, tag="ident")
            nc.sync.dma_start(id_s[:], ident[:])

            myshardA = dpool.tile([HALFROWS, HID], dt.bfloat16, tag="myshardA")
            myshardB = dpool.tile([SHARD - HALFROWS, HID], dt.bfloat16,
                                  tag="myshardB")
            tableA = dpool.tile([TBL_A, HID], dt.bfloat16, tag="tableA")
            tableB = dpool.tile([TBL_B, HID], dt.bfloat16, tag="tableB")

            h_prev = None
            for layer in range(3):
                # ---- dense: T = h @ W  (feat-major PSUM [128, chunk]) ----
                tbf = tpool.tile([HID, DPAD], dt.bfloat16, tag="tbf")
                col = 0
                sizes = [512] * 12 + [128]
                for sz in sizes:
                    pd = pdpool.tile([HID, 512], dt.float32, tag="pd")
                    if layer == 0:
                        nc.tensor.matmul(pd[:, :sz], W1_s[:],
                                         xT_s[:, col:col + sz],
                                         start=True, stop=True)
                    else:
                        W_s = W2_s if layer == 1 else W3_s
                        nc.tensor.matmul(pd[:, :sz], W_s[:],
                                         h_prev[:, col:col + sz],
                                         start=True, stop=True)
                    nc.scalar.activation(tbf[:, col:col + sz], pd[:, :sz],
                                         mybir.ActivationFunctionType.Copy)
                    col += sz

                # ---- transpose local shard to node-major rows ----
                stage = spool.tile([128, NTILE, HID], dt.bfloat16, tag="stage")
                for t in range(NTILE):
                    pt = ptpool.tile([128, 128], dt.bfloat16, tag="pt")
                    nc.tensor.transpose(pt[:], tbf[:, t * 128:(t + 1) * 128],
                                        id_s[:])
                    nc.vector.tensor_copy(stage[:, t, :], pt[:])
                # ---- spill to DRAM half-shards + two allgathers ----
                for t in range(NTILE_A):
                    nc.sync.dma_start(myshardA[t * 128:(t + 1) * 128, :],
                                      stage[:, t, :])
                nc.gpsimd.collective_compute(
                    "AllGather",
                    mybir.AluOpType.bypass,
                    ins=[myshardA.opt()],
                    outs=[tableA.opt()],
                    replica_groups=[list(range(NCORES))],
                )
                for t in range(NTILE_A, NTILE - 1):
                    nc.sync.dma_start(
                        myshardB[t * 128 - HALFROWS:(t + 1) * 128 - HALFROWS, :],
                        stage[:, t, :])
                nc.sync.dma_start(myshardB[(NTILE - 1) * 128 - HALFROWS:
                                           SHARD - HALFROWS, :],
                                  stage[:SHARD - (NTILE - 1) * 128, NTILE - 1, :])
                nc.gpsimd.collective_compute(
                    "AllGather",
                    mybir.AluOpType.bypass,
                    ins=[myshardB.opt()],
                    outs=[tableB.opt()],
                    replica_groups=[list(range(NCORES))],
                )

                # ---- gather + scatter-matmul + relu evict ----
                h_next = hpool.tile([HID, DPAD], dt.bfloat16, tag="h")
                o0 = o1 = 0
                for k in range(CHUNKS):
                    meta = chunk_meta[k]
                    n0, n1, ctot = meta["n0"], meta["n1"], meta["ctot"]
                    msg = mpool.tile([128, cmax, HID], dt.bfloat16, tag="msg")
                    qq = 2 * k
                    for base, n_h, tb, ix, oo in ((0, n0, tableA, idx0_s, o0),
                                                  (n0, n1, tableB, idx1_s, o1)):
                        nsub = (n_h // 256) * 128
                        for s0, sn in ((0, nsub), (nsub, n_h - nsub)):
                            if not sn:
                                continue
                            nc.gpsimd.dma_gather(
                                out_ap=msg[:, (base + s0) // 128:
                                           (base + s0 + sn) // 128, :],
                                in_ap=tb[:],
                                idxs_ap=ix[:, (oo + s0) // 16:
                                           (oo + s0 + sn) // 16],
                                num_idxs=sn, num_idxs_reg=sn,
                                elem_size=HID, elem_step=HID,
                                single_packet=False, queue_num=qq % 4)
                            qq += 1
                    o0 += n0
                    o1 += n1
                    blk0 = meta["sched"][0][2]
                    nblk = len(meta["sched"])
                    ind_c = ipool.tile([128, bmax * WIN], dt.bfloat16,
                                       tag="indc")
                    nc.sync.dma_start(
                        ind_c[:, :nblk * WIN],
                        indt[:, blk0 * WIN:(blk0 + nblk) * WIN])
                    ps = pspool.tile([HID, CHUNK_WINS * WIN], dt.float32,
                                     tag="ps")
                    for (wi, brow, blk, first, last) in meta["sched"]:
                        nc.tensor.matmul(
                            ps[:, wi * WIN:(wi + 1) * WIN],
                            msg[:, brow, :],
                            ind_c[:, (blk - blk0) * WIN:(blk - blk0 + 1) * WIN],
                            start=first, stop=last)
                    nc.scalar.activation(
                        h_next[:, k * CHUNK_WINS * WIN:(k + 1) * CHUNK_WINS * WIN],
                        ps[:],
                        mybir.ActivationFunctionType.Relu,
                        bias=b_s[:, layer:layer + 1])
                h_prev = h_next

            # ---- partial mean-pool over local nodes ----
            gs = cpool.tile([HID, 1], dt.float32, tag="gs")
            nc.vector.tensor_reduce(gs[:], h_prev[:, :SHARD],
                                    axis=mybir.AxisListType.X,
                                    op=mybir.AluOpType.add)
            nc.sync.dma_start(gsum[:], gs[:])

    nc.compile()
    return nc


def _heads(g, W, b, W2_, b2_):
    h = np.maximum(g @ W + b, 0.0)
    return h @ W2_ + b2_


def kernel(x, edge_index, W1, b1, W2, b2, W3, b3,
           Wv1, bv1, Wv2, bv2, Wa1, ba1, Wa2, ba2):
    from concourse.bass_utils import run_bass_kernel_spmd
    try:
        import axon_profile
        axon_profile.install()
    except Exception:
        pass

    x = np.asarray(x, dtype=np.float32)
    prep = _prep(np.asarray(edge_index))
    nc = _build(prep)

    in_maps = []
    for c in range(NCORES):
        pc = prep["per_core"][c]
        xT_loc = np.zeros((F_IN, DPAD), dtype=np.float32)
        xT_loc[:, :SHARD] = x[c * SHARD:(c + 1) * SHARD, :].T
        in_maps.append({
            "xT": xT_loc,
            "W1": np.asarray(W1, dtype=np.float32),
            "W2": np.asarray(W2, dtype=np.float32).astype(bf16),
            "W3": np.asarray(W3, dtype=np.float32).astype(bf16),
            "bcols": np.stack([np.asarray(b1, np.float32),
                               np.asarray(b2, np.float32),
                               np.asarray(b3, np.float32)], axis=1),
            "ident": np.eye(128, dtype=bf16),
            "idx0": pc["idx0"],
            "idx1": pc["idx1"],
            "indt": pc["ind"],
        })

    import os
    trace = bool(os.environ.get("KERNEL_TRACE"))
    res = run_bass_kernel_spmd(nc, in_maps, core_ids=list(range(NCORES)),
                               trace=trace)
    global LAST_RESULT
    LAST_RESULT = res
    if trace and res.exec_time_ns is not None:
        print(f"HW exec time: {res.exec_time_ns} ns")
    partial = np.stack([r["gsum"][:, 0] for r in res.results])  # [8, 128]
    g = (partial.sum(axis=0) / N_NODES).astype(np.float32)[None, :]  # [1, 128]

    value = _heads(g, np.asarray(Wv1, np.float32), np.asarray(bv1, np.float32),
                   np.asarray(Wv2, np.float32), np.asarray(bv2, np.float32))
    adv = _heads(g, np.asarray(Wa1, np.float32), np.asarray(ba1, np.float32),
                 np.asarray(Wa2, np.float32), np.asarray(ba2, np.float32))
    q = value + (adv - adv.mean(axis=-1, keepdims=True))
    return q.astype(np.float32)
